# revision 1
# baseline (speedup 1.0000x reference)
"""Per-edge dot product score[e] = h[src[e]] . h[dst[e]] on 8 TRN2 NeuronCores.

Design (per core, edges sharded 8 ways):
 - Host pads h to a 256B-pitch table h_pad [N, 64] f32 (dma_gather's DRAM
   stride must be a multiple of 256B; the gather payload itself is the
   first 128B = 32 f32 of each row).
 - Host sorts the core's 200k edges by (src_chunk, dst_chunk) where a
   chunk is 25k nodes (dma_gather indices are int16), giving 16 groups
   padded to a fixed capacity C with -1 (desc-gen skips the tail; the
   true count rides in a runtime register). Index streams are pre-wrapped
   into the ucode's [16, C/16] layout and replicated across 8 Q7 groups.
 - Device: per group, two InstDMAGatherAnt calls (custom GPSIMD ucode,
   one 128B descriptor per edge endpoint) fetch h rows for src and dst
   into [128, C/128, 32] SBUF tiles. Descriptor generation is the
   bottleneck (~10 ns/row per SWDGE queue), so gathers round-robin across
   all 4 SWDGE queues and run 4 groups deep (8 calls in flight) for ~4x
   parallel descriptor generation. DVE multiplies and reduces over the
   32 features; scores stream back to DRAM.
 - Host inverse-permutes the scores back to original edge order.
"""

import numpy as np

# problem shape
N_NODES = 100000
D = 32
N_EDGES = 1600000
N_CORES = 8
E_PC = N_EDGES // N_CORES      # 200000

# kernel tiling
P = 128
N_CHUNKS = 4                   # int16 index windows over the node table
NPC = 25000                    # nodes per chunk
G = N_CHUNKS * N_CHUNKS        # 16 sort groups
C = 13056                      # edge capacity per group (= 128*102)
HP = 64                        # padded row width (f32) -> 256B pitch
NSLOT = 4                      # pipeline depth (groups in flight)
SPLIT = 1                      # sub-calls per group per side (64KB Q7 scratch fits 13312*4B)
RUNTIME_COUNTS = True          # skip -1 tail descriptors via runtime register

_CACHE = {}


def _dma_gather_raw(g, out_ap, in_ap, idxs_ap, num_idxs, num_idxs_reg,
                    elem_size, elem_step, queue_num):
    """bass.dma_gather minus the elem_size%256 assert (the 256B constraint
    is on the DRAM stride, encoded in 256B units; a 128B half-row payload
    per descriptor is accepted by the ucode, verified on HW)."""
    from concourse import ap_utils, mybir
    from concourse.bass import round_up_to_multiple

    g._assert_queue_num(queue_num)
    assert idxs_ap.dtype == mybir.dt.int16
    assert in_ap.dtype == out_ap.dtype
    assert ap_utils.ap_is_contiguous(in_ap.ap[1:])
    assert ap_utils.ap_is_contiguous(out_ap.ap[1:])
    assert ap_utils.ap_is_contiguous(idxs_ap.ap[1:])
    assert num_idxs % 4 == 0
    assert in_ap.ap[-1][1] == elem_size and out_ap.ap[-1][1] == elem_size
    assert out_ap.ap[0][1] * out_ap.ap[1][1] == round_up_to_multiple(num_idxs, 128)
    assert in_ap.ap[0][0] == elem_step
    stride_bytes = elem_step * mybir.dt.size(in_ap.dtype)
    assert stride_bytes % 256 == 0 and stride_bytes // 256 < 256
    _in_ap = g.lower_ap_dma(in_ap, for_custom_bir_dma=True)
    _idxs_ap = g.lower_ap(idxs_ap)
    _out_ap = g.lower_ap(out_ap)
    return g.add_instruction(
        mybir.InstDMAGatherAnt(
            name=g.bass.get_next_instruction_name(),
            ins=[*_in_ap, _idxs_ap, g.lower_val_access(g.to_reg(num_idxs_reg))],
            outs=[_out_ap],
            transpose=False,
            num_idxs=num_idxs,
            elem_size=elem_size,
            stride_bytes_256=stride_bytes // 256,
            gen_mode=0,
            single_packet=False,
            queue_num=queue_num,
            sbuf_tokens_per_rank=0,
            sbuf_free_dim_per_rank=0,
            sbuf_free_dim_pad_per_rank=0,
            sbuf_byte_offset=0,
        )
    )


def _build(cap=None):
    from contextlib import ExitStack

    import concourse.bacc as bacc
    import concourse.bass as bass
    from concourse import mybir
    from concourse.library_config import mlp

    cap = C if cap is None else cap
    COLS = cap // P
    W = cap // 16

    nc = bacc.Bacc("TRN2", target_bir_lowering=False, debug=False,
                   num_swdge_queues=4)

    h_pad = nc.dram_tensor("h_pad", [N_NODES, HP], mybir.dt.float32,
                           kind="ExternalInput")
    idx_src = nc.dram_tensor("idx_src", [G, P, W], mybir.dt.int16,
                             kind="ExternalInput")
    idx_dst = nc.dram_tensor("idx_dst", [G, P, W], mybir.dt.int16,
                             kind="ExternalInput")
    cnt = nc.dram_tensor("cnt", [1, SPLIT * G], mybir.dt.int32, kind="ExternalInput")
    score = nc.dram_tensor("score", [G, P, COLS], mybir.dt.float32,
                           kind="ExternalOutput")

    def chunk_ap(c):
        return h_pad[c * NPC:(c + 1) * NPC, :D]

    with (
        nc.Block() as block,
        nc.sbuf_tensor("ixs", [P, NSLOT, W], mybir.dt.int16) as ixs,
        nc.sbuf_tensor("ixd", [P, NSLOT, W], mybir.dt.int16) as ixd,
        nc.sbuf_tensor("hs", [P, NSLOT, COLS, D], mybir.dt.float32) as hs,
        nc.sbuf_tensor("hd", [P, NSLOT, COLS, D], mybir.dt.float32) as hd,
        nc.sbuf_tensor("sc", [P, NSLOT, COLS], mybir.dt.float32) as sc,
        nc.sbuf_tensor("cnt_sb", [1, SPLIT * G], mybir.dt.int32) as cnt_sb,
        nc.semaphore("dve_sem") as dve_sem,
        nc.semaphore("mr_sem") as mr_sem,
        nc.semaphore("cnt_sem") as cnt_sem,
        ExitStack() as stack,
    ):
        qs = [stack.enter_context(nc.semaphore(f"q{i}")) for i in range(2 * SPLIT * NSLOT)]  # noqa: ANT232
        ix_sem = [stack.enter_context(nc.semaphore(f"ix{i}")) for i in range(NSLOT)]  # noqa: ANT232
        st_sem = [stack.enter_context(nc.semaphore(f"st{i}")) for i in range(NSLOT)]  # noqa: ANT232

        @block.sync
        def _(sp: bass.BassEngine):
            sp.dma_start(cnt_sb[:], cnt[:]).then_inc(cnt_sem, 16)
            # prologue: index tiles for the first NSLOT groups
            for g in range(NSLOT):
                sp.dma_start(ixs[:, g], idx_src[g]).then_inc(ix_sem[g], 16)
                sp.dma_start(ixd[:, g], idx_dst[g]).then_inc(ix_sem[g], 16)
            for g in range(G):
                sp.wait_ge(dve_sem, g + 1)
                sp.dma_start(score[g], sc[:, g % NSLOT]).then_inc(st_sem[g % NSLOT], 16)
                if g + NSLOT < G:
                    # safe: dve_sem>=g+1 implies group g's desc-gen read
                    # its index tiles, so buffer g%NSLOT is reusable
                    sp.dma_start(ixs[:, g % NSLOT],
                                 idx_src[g + NSLOT]).then_inc(ix_sem[g % NSLOT], 16)
                    sp.dma_start(ixd[:, g % NSLOT],
                                 idx_dst[g + NSLOT]).then_inc(ix_sem[g % NSLOT], 16)

        @block.gpsimd
        def _(gp: bass.BassGpSimd):
            gp.load_library(mlp)
            gp.wait_ge(cnt_sem, 16)
            cnt_regs = [gp.alloc_register(f"cnt_reg{i}") for i in range(SPLIT * NSLOT)]
            for g in range(G):
                a, b = g // N_CHUNKS, g % N_CHUNKS
                s = g % NSLOT
                gp.wait_ge(ix_sem[s], 32 * (g // NSLOT + 1))
                if g >= NSLOT:
                    gp.wait_ge(dve_sem, g - NSLOT + 1)   # gather buf s consumed
                if RUNTIME_COUNTS:
                    # dedicated rotating registers: the Q7 queue worker reads
                    # the count register asynchronously; reuse distance of
                    # NSLOT groups guarantees the prior reader is done
                    for q in range(SPLIT):
                        gp.reg_load(cnt_regs[SPLIT * s + q],
                                    cnt_sb[0:1, SPLIT * g + q:SPLIT * g + q + 1])
                part = cap // SPLIT
                for j, (buf, ix, ch) in enumerate(
                        ((hs, ixs, a), (hd, ixd, b))):
                    for q in range(SPLIT):
                        c = 2 * SPLIT * g + SPLIT * j + q
                        _dma_gather_raw(
                            gp,
                            buf[:, s, q * (COLS // SPLIT):(q + 1) * (COLS // SPLIT)],
                            chunk_ap(ch),
                            ix[:, s, q * (W // SPLIT):(q + 1) * (W // SPLIT)],
                            part,
                            cnt_regs[SPLIT * s + q] if RUNTIME_COUNTS else part,
                            D, HP, queue_num=c % 4,
                        ).then_inc(qs[c % (2 * SPLIT * NSLOT)], 16)

        @block.vector
        def _(v: bass.BassEngine):
            for g in range(G):
                s = g % NSLOT
                nsem = 2 * SPLIT * NSLOT
                for c in range(2 * SPLIT * g, 2 * SPLIT * (g + 1)):
                    v.wait_ge(qs[c % nsem], 16 * (c // nsem + 1))
                if g >= NSLOT:
                    v.wait_ge(st_sem[s], 16 * (g // NSLOT))   # sc buf s stored
                v.tensor_mul(hs[:, s], hs[:, s], hd[:, s]).then_inc(mr_sem, 1)
                v.wait_ge(mr_sem, g + 1)
                v.tensor_reduce(
                    sc[:, s], hs[:, s], axis=mybir.AxisListType.X,
                    op=mybir.AluOpType.add,
                ).then_inc(dve_sem, 1)

    nc.compile()
    return nc


def _get_nc(cap=None):
    cap = C if cap is None else cap
    key = ("nc", cap)
    if key not in _CACHE:
        _CACHE[key] = _build(cap)
    return _CACHE[key]


def _prep(h, src, dst, cap):
    """Host-side marshaling: pad h, sort each core's edges into the 16
    (src_chunk, dst_chunk) groups, wrap indices, build inverse perms."""
    W = cap // 16
    h = np.asarray(h, dtype=np.float32)
    src = np.asarray(src).astype(np.int64)
    dst = np.asarray(dst).astype(np.int64)

    h_pad = np.zeros((N_NODES, HP), dtype=np.float32)
    h_pad[:, :D] = h

    in_maps, perms = [], []
    for c in range(N_CORES):
        s = src[c * E_PC:(c + 1) * E_PC]
        d = dst[c * E_PC:(c + 1) * E_PC]
        ga = s // NPC
        gb = d // NPC
        grp = ga * N_CHUNKS + gb
        order = np.argsort(grp, kind="stable")
        counts = np.bincount(grp, minlength=G)
        if counts.max() > cap:
            raise _Overflow(int(counts.max()))
        sloc = (s - ga * NPC)[order].astype(np.int16)
        dloc = (d - gb * NPC)[order].astype(np.int16)

        # effective counts: at least 16, rounded up to a multiple of 16
        # (desc-gen truncates the index stream at the last non-negative
        # entry in 16-wrapped units); pad [true, eff) with 0, rest -1
        # split each group's edges into SPLIT equal shares so every SWDGE
        # queue carries the same row count (call->queue is static per
        # call-type; unequal halves would overload half the queues)
        part = cap // SPLIT
        qcnt = np.zeros((G, SPLIT), dtype=np.int64)
        src16 = np.full((G, cap), -1, dtype=np.int16)
        dst16 = np.full((G, cap), -1, dtype=np.int16)
        perm = np.full((G, cap), -1, dtype=np.int64)
        offs = np.concatenate([[0], np.cumsum(counts)])
        for gi in range(G):
            n = counts[gi]
            gs = sloc[offs[gi]:offs[gi] + n]
            gd = dloc[offs[gi]:offs[gi] + n]
            go = order[offs[gi]:offs[gi] + n]
            base = n // SPLIT
            shares = [base + (1 if q < n % SPLIT else 0) for q in range(SPLIT)]
            done = 0
            for q in range(SPLIT):
                nq = shares[q]
                lo = q * part
                e = int(np.clip((nq + 15) // 16 * 16, 16, part))
                qcnt[gi, q] = e
                src16[gi, lo:lo + nq] = gs[done:done + nq]
                dst16[gi, lo:lo + nq] = gd[done:done + nq]
                src16[gi, lo + nq:lo + e] = 0
                dst16[gi, lo + nq:lo + e] = 0
                perm[gi, lo:lo + nq] = go[done:done + nq]
                done += nq

        # wrap to the ucode layout [16, cap/16] and replicate across the
        # 8 Q7 16-partition groups -> [128, W]
        def wrap(x):
            w = x.reshape(G, W, 16).transpose(0, 2, 1)       # [G, 16, W]
            w = np.broadcast_to(w[:, None], (G, 8, 16, W))
            return np.ascontiguousarray(w.reshape(G, P, W))

        in_maps.append({
            "h_pad": h_pad,
            "idx_src": wrap(src16),
            "idx_dst": wrap(dst16),
            "cnt": np.ascontiguousarray(qcnt.reshape(1, SPLIT * G)).astype(np.int32),
        })
        perms.append(perm.reshape(-1))
    return in_maps, perms


class _Overflow(Exception):
    def __init__(self, n):
        super().__init__(f"group overflow: {n}")
        self.n = n


def run(h, src, dst, trace=False):
    """Returns (score [N_EDGES, 1] float32, exec_time_ns or None)."""
    from concourse.bass_utils import run_bass_kernel_spmd

    cap = C
    try:
        in_maps, perms = _prep(h, src, dst, cap)
    except _Overflow as e:
        # pathological (non-uniform) edge distribution: recompile with a
        # capacity that fits
        cap = (e.n + 255) // 256 * 256   # %256: halves stay 128-aligned
        in_maps, perms = _prep(h, src, dst, cap)
    nc = _get_nc(cap)
    res = run_bass_kernel_spmd(nc, in_maps, list(range(N_CORES)), trace=trace)
    _CACHE["last_res"] = res
    cols = cap // P
    out = np.empty(N_EDGES, dtype=np.float32)
    for c in range(N_CORES):
        sc = res.results[c]["score"]                 # [G, P, cols]
        flat = sc.transpose(0, 2, 1).reshape(-1)     # padded pos g*cap + col*128 + p
        perm = perms[c]
        valid = perm >= 0
        out[c * E_PC + perm[valid]] = flat[valid]
    return out.reshape(N_EDGES, 1), res.exec_time_ns


def kernel(h, src, dst):
    out, _ = run(h, src, dst, trace=False)
    return out

